# revision 18
# baseline (speedup 1.0000x reference)
"""Trainium2 Bass kernel for CCPLoss:
out = sigmoid(mean(|maxpool35(min_c restored) - maxpool35(min_c target)|))

Inputs: restored, target: [16, 3, 512, 512] fp32.
Sharding: pure data parallel over batch; 2 images per core on 8 cores.
Per-core partial |diff| sums are reduced on host, then mean+sigmoid on host.

Per image:
 - SWDGE cast DMA loads fp32->bf16 (all 3 channels, one DMA).
 - channel-min: first min on GpSimd, second on DVE (writes into padded tile).
 - separable 35x35 stride-1 max pool, zero padding (data >= 0, so 0 == -inf
   here): van Herk via DVE tensor_tensor_scan (masked running max, forward +
   reverse along the free dim) + one shifted tensor_tensor max per axis.
 - W pass in natural layout; PE transpose (identity matmul, 128x128 blocks,
   bf16 PSUM) + ACT copies into padded layout; H pass on transposed data.
Tail per pair: diff (DVE/GpSimd), ACT Abs with fused accumulate; host sums.
"""

import sys

for _p in ("/opt/trn_rl_repo",):
    if _p not in sys.path:
        sys.path.insert(0, _p)

import numpy as np

import concourse.bass as bass
import concourse.mybir as mybir
from concourse import bacc, masks
from concourse.bass_utils import run_bass_kernel_spmd
from concourse.tile import TileContext

F32 = mybir.dt.float32
BF16 = mybir.dt.bfloat16
ALU = mybir.AluOpType

N_CORES = 8
B_FULL = 16
B_PER_CORE = B_FULL // N_CORES  # 2
C = 3
H = W = 512
K = 35
PAD = K // 2  # 17
NB = 546  # padded per-chunk length (512 + 2*17)
NCHUNK = 4  # 512 rows = 4 chunks of 128 partitions
FDP = NCHUNK * NB  # 2184
FD = NCHUNK * W  # 2048

_COMPILED = None


def _build_nc(reps=1, sim_safe=False):
    nc = bacc.Bacc("TRN2", detect_race_conditions=False)
    restored = nc.declare_dram_parameter(
        "restored", [B_PER_CORE, C, H, W], F32, isOutput=False
    )
    target = nc.declare_dram_parameter(
        "target", [B_PER_CORE, C, H, W], F32, isOutput=False
    )
    partial = nc.declare_dram_parameter("partial", [128, 2], F32, isOutput=True)

    with (
        TileContext(nc) as tc,
        tc.tile_pool(name="const", bufs=1) as cpool,
        tc.tile_pool(name="work", bufs=1) as pool,
        tc.tile_pool(name="psum", bufs=4, space="PSUM") as ppool,
    ):
        def load_image(inp, b, who, split=False):
            """SWDGE cast DMA(s): [3,512,512] f32 -> [128, 3*2048] bf16.
            split=True loads channels 0-1 and 2 separately so the first
            channel-min can start before the full image lands."""
            Xc = pool.tile([128, C * FD], BF16, tag="Xc", bufs=4, name=f"Xc_{who}")
            Xc4 = Xc.rearrange("p (ch c w) -> p ch c w", ch=C, w=W)
            src = inp[b].rearrange("ch (c p) w -> p ch c w", p=128)
            if split:
                nc.gpsimd.dma_start(Xc4[:, 0:2], src[:, 0:2])
                nc.gpsimd.dma_start(Xc4[:, 2:3], src[:, 2:3])
            else:
                nc.gpsimd.dma_start(Xc4, src)
            return Xc

        def w_phase(Xc, who, first, mask, maskR):
            """channel-min + W-axis van Herk. Returns Rw [128, FD] bf16."""
            Xc3 = Xc.rearrange("p (ch n) -> p ch n", n=FD)
            nc.vector.tensor_tensor(Xc3[:, 0], Xc3[:, 0], Xc3[:, 1], ALU.min)
            X = pool.tile([128, FDP], BF16, tag="X", bufs=2, name=f"X_{who}")
            X3 = X.rearrange("p (c n) -> p c n", n=NB)
            if first:
                # pads sit at fixed slot addresses; later images reuse the
                # same two physical slots, whose pads stay zero
                nc.vector.memset(X3[:, :, 0:PAD], 0.0)
                nc.vector.memset(X3[:, :, H + PAD : NB], 0.0)
            nc.vector.tensor_tensor(
                X3[:, :, PAD : PAD + W],
                Xc3[:, 0].rearrange("p (c w) -> p c w", w=W),
                Xc3[:, 2].rearrange("p (c w) -> p c w", w=W),
                ALU.min,
            )
            Fw = pool.tile([128, FDP], BF16, tag="F", bufs=2, name=f"F_{who}")
            Gw = pool.tile([128, FDP], BF16, tag="G", bufs=2, name=f"G_{who}")
            nc.vector.tensor_tensor_scan(
                Fw[:], mask[:], X[:], 0.0, ALU.mult, ALU.max
            )
            nc.vector.tensor_tensor_scan(
                Gw[:, ::-1], maskR[:], X[:, ::-1], 0.0, ALU.mult, ALU.max
            )
            Rw = pool.tile([128, FD], BF16, tag="Rw", bufs=4, name=f"Rw_{who}")
            F3 = Fw.rearrange("p (c n) -> p c n", n=NB)
            G3 = Gw.rearrange("p (c n) -> p c n", n=NB)
            R3 = Rw.rearrange("p (c n) -> p c n", n=W)
            nc.vector.tensor_tensor(
                R3[:], G3[:, :, 0:W], F3[:, :, K - 1 : K - 1 + W], ALU.max
            )
            return Rw

        def h_phase(Rw, who, first, mask, maskR, ident):
            """PE transpose + H-axis van Herk. Returns RT [128, FD] bf16."""
            X2 = pool.tile([128, FDP], BF16, tag="X2", bufs=2, name=f"X2_{who}")
            X23 = X2.rearrange("p (d n) -> p d n", n=NB)
            if first:
                nc.vector.memset(X23[:, :, 0:PAD], 0.0)
                nc.vector.memset(X23[:, :, H + PAD : NB], 0.0)
            for d in range(NCHUNK):
                ps = ppool.tile([128, 512], BF16, tag="ps", name=f"ps_{who}_{d}")
                for c2 in range(NCHUNK):
                    nc.tensor.transpose(
                        ps[:, c2 * 128 : (c2 + 1) * 128],
                        Rw[:, c2 * W + d * 128 : c2 * W + d * 128 + 128],
                        ident[:],
                    )
                nc.scalar.copy(X23[:, d, PAD : PAD + H], ps[:])
            Fh = pool.tile([128, FDP], BF16, tag="F2", bufs=2, name=f"F2_{who}")
            Gh = pool.tile([128, FDP], BF16, tag="G2", bufs=2, name=f"G2_{who}")
            nc.vector.tensor_tensor_scan(
                Fh[:], mask[:], X2[:], 0.0, ALU.mult, ALU.max
            )
            nc.vector.tensor_tensor_scan(
                Gh[:, ::-1], maskR[:], X2[:, ::-1], 0.0, ALU.mult, ALU.max
            )
            RT = pool.tile([128, FD], BF16, tag="RT", bufs=2, name=f"RT_{who}")
            F23 = Fh.rearrange("p (d n) -> p d n", n=NB)
            G23 = Gh.rearrange("p (d n) -> p d n", n=NB)
            RT3 = RT.rearrange("p (d n) -> p d n", n=H)
            nc.vector.tensor_tensor(
                RT3[:], G23[:, :, 0:H], F23[:, :, K - 1 : K - 1 + H], ALU.max
            )
            return RT

        smax = None
        for rep in range(reps):
            # issue all loads first so DMA streams ahead of compute
            Xcs = [
                load_image(inp, b, f"{nm}{b}_{rep}", split=(rep == 0 and b == 0))
                for b in range(B_PER_CORE)
                for nm, inp in (("r", restored), ("t", target))
            ]
            if rep == 0:
                # constants after the DMAs in program order: Pool engine gets
                # the descriptor generation out first
                mask = cpool.tile([128, FDP], BF16)
                nc.vector.memset(mask[:], 1.0)
                mask3 = mask.rearrange("p (c n) -> p c n", n=NB)
                nc.vector.memset(mask3[:, :, 0::K], 0.0)
                # reverse-scan mask: block ends map to stream positions
                # 0, 21, 56, 91, ... because NB mod K = 21
                maskR = cpool.tile([128, FDP], BF16)
                nc.vector.memset(maskR[:], 1.0)
                maskR3 = maskR.rearrange("p (c n) -> p c n", n=NB)
                nc.vector.memset(maskR3[:, :, 0:1], 0.0)
                nc.vector.memset(maskR3[:, :, NB % K :: K], 0.0)
                ident = cpool.tile([128, 128], BF16)
                masks.make_identity(nc, ident[:])
                smax = cpool.tile([128, 1], F32)
                nc.vector.memset(smax[:], 0.0)

            first = rep == 0
            Rws = [
                w_phase(Xcs[i], f"i{i}_{rep}", sim_safe or (first and i < 2), mask, maskR)
                for i in range(4)
            ]
            RTs = []
            accs = []
            for p in range(B_PER_CORE):
                Rr = h_phase(Rws[2 * p], f"hr{p}_{rep}", sim_safe or (first and p == 0), mask, maskR, ident)
                Rt = h_phase(Rws[2 * p + 1], f"ht{p}_{rep}", sim_safe or (first and p == 0), mask, maskR, ident)
                scr = pool.tile([128, FD], BF16, tag="scr", bufs=2, name=f"scr{p}_{rep}")
                sabs = pool.tile([128, FD], BF16, tag="sabs", bufs=2, name=f"sabs{p}_{rep}")
                amax = pool.tile([128, 1], F32, tag="amax", bufs=2, name=f"am{p}_{rep}")
                nc.vector.tensor_tensor(scr[:], Rr[:], Rt[:], ALU.subtract)
                nc.scalar.activation(
                    sabs[:], scr[:], mybir.ActivationFunctionType.Abs,
                    accum_out=amax[:],
                )
                accs.append(amax)
            for amax in accs:
                nc.vector.tensor_tensor(smax[:], smax[:], amax[:], ALU.add)

        out2 = pool.tile([128, 2], F32)
        nc.vector.memset(out2[:, 1:2], 0.0)
        nc.vector.tensor_copy(out2[:, 0:1], smax[:])
        nc.sync.dma_start(partial[:], out2[:])

    nc.compile()
    return nc


def _get_compiled():
    global _COMPILED
    if _COMPILED is None:
        _COMPILED = _build_nc()
    return _COMPILED


def kernel(restored: np.ndarray, target: np.ndarray) -> np.ndarray:
    restored = np.ascontiguousarray(restored, dtype=np.float32)
    target = np.ascontiguousarray(target, dtype=np.float32)
    nc = _get_compiled()
    in_maps = []
    for i in range(N_CORES):
        sl = slice(i * B_PER_CORE, (i + 1) * B_PER_CORE)
        in_maps.append(
            {
                "restored": np.ascontiguousarray(restored[sl]),
                "target": np.ascontiguousarray(target[sl]),
            }
        )
    res = run_bass_kernel_spmd(nc, in_maps, list(range(N_CORES)))
    total = np.float64(0.0)
    for r in res.results:
        p = np.asarray(r["partial"], dtype=np.float64)
        total += p[:, 0].sum() - p[:, 1].sum()
    mean = total / float(B_FULL * H * W)
    out = 1.0 / (1.0 + np.exp(-mean))
    return np.asarray(out, dtype=np.float32)


# revision 19
# speedup vs baseline: 1.3956x; 1.3956x over previous
"""Trainium2 Bass kernel for CCPLoss:
out = sigmoid(mean(|maxpool35(min_c restored) - maxpool35(min_c target)|))

Inputs: restored, target: [16, 3, 512, 512] fp32.
Sharding: pure data parallel over batch; 2 images per core on 8 cores.
Per-core partial |diff| sums are reduced on host, then mean+sigmoid on host.

Per image:
 - SWDGE cast DMA loads fp32->bf16 (3 channels at once).
 - channel-min: two bf16 tensor_tensor mins on DVE (2x mode).
 - separable 35x35 stride-1 max pool with zero padding (data >= 0, so 0
   behaves as -inf): shift-max doubling along the free dim, shifts
   {1,2,4,8,16,3} (subset sums cover 0..34), six 2x bf16 tensor_tensor max
   ops per axis, ping-pong buffers.
 - W pass in natural layout; PE transpose (identity matmul, 128x128 blocks,
   bf16 PSUM) + ACT copies into padded layout; H pass on transposed data.
Tail per pair: diff on DVE, ACT Abs with fused accumulate; host sums.
"""

import sys

for _p in ("/opt/trn_rl_repo",):
    if _p not in sys.path:
        sys.path.insert(0, _p)

import numpy as np

import concourse.bass as bass
import concourse.mybir as mybir
from concourse import bacc, masks
from concourse.bass_utils import run_bass_kernel_spmd
from concourse.tile import TileContext

F32 = mybir.dt.float32
BF16 = mybir.dt.bfloat16
ALU = mybir.AluOpType

N_CORES = 8
B_FULL = 16
B_PER_CORE = B_FULL // N_CORES  # 2
C = 3
H = W = 512
K = 35
PAD = K // 2  # 17
NB = 546  # padded per-chunk length (512 + 2*17)
NCHUNK = 4  # 512 rows = 4 chunks of 128 partitions
FDP = NCHUNK * NB  # 2184
FD = NCHUNK * W  # 2048
SHIFTS = (1, 2, 4, 8, 16, 3)  # subset sums cover 0..34

_COMPILED = None


def _build_nc(reps=1, sim_safe=False):
    nc = bacc.Bacc("TRN2", detect_race_conditions=False)
    restored = nc.declare_dram_parameter(
        "restored", [B_PER_CORE, C, H, W], F32, isOutput=False
    )
    target = nc.declare_dram_parameter(
        "target", [B_PER_CORE, C, H, W], F32, isOutput=False
    )
    partial = nc.declare_dram_parameter("partial", [128, 2], F32, isOutput=True)

    with (
        TileContext(nc) as tc,
        tc.tile_pool(name="const", bufs=1) as cpool,
        tc.tile_pool(name="work", bufs=1) as pool,
        tc.tile_pool(name="psum", bufs=4, space="PSUM") as ppool,
    ):
        def load_image(inp, b, who, split=False):
            """SWDGE cast DMA(s): [3,512,512] f32 -> [128, 3*2048] bf16.
            split=True loads channels 0-1 and 2 separately so the first
            channel-min can start before the full image lands."""
            Xc = pool.tile([128, C * FD], BF16, tag="Xc", bufs=4, name=f"Xc_{who}")
            Xc4 = Xc.rearrange("p (ch c w) -> p ch c w", ch=C, w=W)
            src = inp[b].rearrange("ch (c p) w -> p ch c w", p=128)
            if split:
                nc.gpsimd.dma_start(Xc4[:, 0:2], src[:, 0:2])
                nc.gpsimd.dma_start(Xc4[:, 2:3], src[:, 2:3])
            else:
                nc.gpsimd.dma_start(Xc4, src)
            return Xc

        def maxpool_1d(Xp, out, who):
            """Sliding-window-35 max along the free dim of the padded
            [128, NCHUNK, NB] view Xp; writes [128, NCHUNK, W] into out.
            Zero pads double as -inf (all data >= 0)."""
            A = pool.tile([128, FDP], BF16, tag="A", bufs=2, name=f"A_{who}")
            Bt = pool.tile([128, FDP], BF16, tag="B", bufs=2, name=f"B_{who}")
            A3 = A.rearrange("p (c n) -> p c n", n=NB)
            B3 = Bt.rearrange("p (c n) -> p c n", n=NB)
            bufs = [Xp, A3, B3, A3, B3, A3]
            cov = 1  # current window length
            for j, s in enumerate(SHIFTS):
                src3 = bufs[j]
                dst3 = out if j == len(SHIFTS) - 1 else bufs[j + 1]
                span = W if j == len(SHIFTS) - 1 else NB - cov - s + 1
                nc.vector.tensor_tensor(
                    dst3[:, :, 0:span],
                    src3[:, :, 0:span],
                    src3[:, :, s : s + span],
                    ALU.max,
                )
                cov += s
            assert cov == K

        def w_phase(Xc, who, first):
            """channel-min + W-axis pool. Returns Rw [128, FD] bf16."""
            Xc3 = Xc.rearrange("p (ch n) -> p ch n", n=FD)
            nc.vector.tensor_tensor(Xc3[:, 0], Xc3[:, 0], Xc3[:, 1], ALU.min)
            X = pool.tile([128, FDP], BF16, tag="X", bufs=2, name=f"X_{who}")
            X3 = X.rearrange("p (c n) -> p c n", n=NB)
            if first:
                # pads sit at fixed slot addresses; later images reuse the
                # same two physical slots, whose pads stay zero
                nc.vector.memset(X3[:, :, 0:PAD], 0.0)
                nc.vector.memset(X3[:, :, H + PAD : NB], 0.0)
            nc.vector.tensor_tensor(
                X3[:, :, PAD : PAD + W],
                Xc3[:, 0].rearrange("p (c w) -> p c w", w=W),
                Xc3[:, 2].rearrange("p (c w) -> p c w", w=W),
                ALU.min,
            )
            Rw = pool.tile([128, FD], BF16, tag="Rw", bufs=4, name=f"Rw_{who}")
            maxpool_1d(X3, Rw.rearrange("p (c n) -> p c n", n=W), f"w{who}")
            return Rw

        def h_phase(Rw, who, first, ident):
            """PE transpose + H-axis pool. Returns RT [128, FD] bf16."""
            X2 = pool.tile([128, FDP], BF16, tag="X2", bufs=2, name=f"X2_{who}")
            X23 = X2.rearrange("p (d n) -> p d n", n=NB)
            if first:
                nc.vector.memset(X23[:, :, 0:PAD], 0.0)
                nc.vector.memset(X23[:, :, H + PAD : NB], 0.0)
            for d in range(NCHUNK):
                ps = ppool.tile([128, 512], BF16, tag="ps", name=f"ps_{who}_{d}")
                for c2 in range(NCHUNK):
                    nc.tensor.transpose(
                        ps[:, c2 * 128 : (c2 + 1) * 128],
                        Rw[:, c2 * W + d * 128 : c2 * W + d * 128 + 128],
                        ident[:],
                    )
                nc.scalar.copy(X23[:, d, PAD : PAD + H], ps[:])
            RT = pool.tile([128, FD], BF16, tag="RT", bufs=2, name=f"RT_{who}")
            maxpool_1d(X23, RT.rearrange("p (d n) -> p d n", n=H), f"h{who}")
            return RT

        smax = None
        for rep in range(reps):
            # issue all loads first so DMA streams ahead of compute
            Xcs = [
                load_image(inp, b, f"{nm}{b}_{rep}", split=(rep == 0 and b == 0))
                for b in range(B_PER_CORE)
                for nm, inp in (("r", restored), ("t", target))
            ]
            if rep == 0:
                ident = cpool.tile([128, 128], BF16)
                masks.make_identity(nc, ident[:])
                smax = cpool.tile([128, 1], F32)
                nc.vector.memset(smax[:], 0.0)

            first = rep == 0
            Rws = [
                w_phase(Xcs[i], f"i{i}_{rep}", sim_safe or (first and i < 2))
                for i in range(4)
            ]
            accs = []
            for p in range(B_PER_CORE):
                Rr = h_phase(
                    Rws[2 * p], f"hr{p}_{rep}", sim_safe or (first and p == 0), ident
                )
                Rt = h_phase(
                    Rws[2 * p + 1], f"ht{p}_{rep}", sim_safe or (first and p == 0), ident
                )
                scr = pool.tile([128, FD], BF16, tag="scr", bufs=2, name=f"scr{p}_{rep}")
                sabs = pool.tile([128, FD], BF16, tag="sabs", bufs=2, name=f"sabs{p}_{rep}")
                amax = pool.tile([128, 1], F32, tag="amax", bufs=2, name=f"am{p}_{rep}")
                nc.vector.tensor_tensor(scr[:], Rr[:], Rt[:], ALU.subtract)
                nc.scalar.activation(
                    sabs[:], scr[:], mybir.ActivationFunctionType.Abs,
                    accum_out=amax[:],
                )
                accs.append(amax)
            for amax in accs:
                nc.vector.tensor_tensor(smax[:], smax[:], amax[:], ALU.add)

        out2 = pool.tile([128, 2], F32)
        nc.vector.memset(out2[:, 1:2], 0.0)
        nc.vector.tensor_copy(out2[:, 0:1], smax[:])
        nc.sync.dma_start(partial[:], out2[:])

    nc.compile()
    return nc


def _get_compiled():
    global _COMPILED
    if _COMPILED is None:
        _COMPILED = _build_nc()
    return _COMPILED


def kernel(restored: np.ndarray, target: np.ndarray) -> np.ndarray:
    restored = np.ascontiguousarray(restored, dtype=np.float32)
    target = np.ascontiguousarray(target, dtype=np.float32)
    nc = _get_compiled()
    in_maps = []
    for i in range(N_CORES):
        sl = slice(i * B_PER_CORE, (i + 1) * B_PER_CORE)
        in_maps.append(
            {
                "restored": np.ascontiguousarray(restored[sl]),
                "target": np.ascontiguousarray(target[sl]),
            }
        )
    res = run_bass_kernel_spmd(nc, in_maps, list(range(N_CORES)))
    total = np.float64(0.0)
    for r in res.results:
        p = np.asarray(r["partial"], dtype=np.float64)
        total += p[:, 0].sum() - p[:, 1].sum()
    mean = total / float(B_FULL * H * W)
    out = 1.0 / (1.0 + np.exp(-mean))
    return np.asarray(out, dtype=np.float32)


# revision 20
# speedup vs baseline: 1.5258x; 1.0933x over previous
"""Trainium2 Bass kernel for CCPLoss:
out = sigmoid(mean(|maxpool35(min_c restored) - maxpool35(min_c target)|))

Inputs: restored, target: [16, 3, 512, 512] fp32.
Sharding: pure data parallel over batch; 2 images per core on 8 cores.
Per-core partial |diff| sums are reduced on host, then mean+sigmoid on host.

Per image:
 - SWDGE cast DMA loads fp32->bf16 (3 channels at once).
 - channel-min: two bf16 tensor_tensor mins on DVE (2x mode).
 - separable 35x35 stride-1 max pool with zero padding (data >= 0, so 0
   behaves as -inf): shift-max doubling along the free dim, shifts
   {1,2,4,8,16,3} (subset sums cover 0..34), six 2x bf16 tensor_tensor max
   ops per axis, ping-pong buffers.
 - W pass in natural layout; PE transpose (identity matmul, 128x128 blocks,
   bf16 PSUM) + ACT copies into padded layout; H pass on transposed data.
Tail per pair: diff on DVE, ACT Abs with fused accumulate; host sums.
"""

import sys

for _p in ("/opt/trn_rl_repo",):
    if _p not in sys.path:
        sys.path.insert(0, _p)

import numpy as np

import concourse.bass as bass
import concourse.mybir as mybir
from concourse import bacc, masks
from concourse.bass_utils import run_bass_kernel_spmd
from concourse.tile import TileContext

F32 = mybir.dt.float32
BF16 = mybir.dt.bfloat16
ALU = mybir.AluOpType

N_CORES = 8
B_FULL = 16
B_PER_CORE = B_FULL // N_CORES  # 2
C = 3
H = W = 512
K = 35
PAD = K // 2  # 17
NB = 546  # padded per-chunk length (512 + 2*17)
NCHUNK = 4  # 512 rows = 4 chunks of 128 partitions
FDP = NCHUNK * NB  # 2184
FD = NCHUNK * W  # 2048
SHIFTS = (1, 2, 4, 8, 16, 3)  # subset sums cover 0..34

_COMPILED = None


def _build_nc(reps=1, sim_safe=False):
    nc = bacc.Bacc("TRN2", detect_race_conditions=False)
    restored = nc.declare_dram_parameter(
        "restored", [B_PER_CORE, C, H, W], F32, isOutput=False
    )
    target = nc.declare_dram_parameter(
        "target", [B_PER_CORE, C, H, W], F32, isOutput=False
    )
    partial = nc.declare_dram_parameter("partial", [128, 2], F32, isOutput=True)

    with (
        TileContext(nc) as tc,
        tc.tile_pool(name="const", bufs=1) as cpool,
        tc.tile_pool(name="work", bufs=1) as pool,
        tc.tile_pool(name="psum", bufs=4, space="PSUM") as ppool,
    ):
        def load_image(inp, b, who, split=False):
            """SWDGE cast DMA(s): [3,512,512] f32 -> [128, 3*2048] bf16.
            split=True loads channels 0-1 and 2 separately so the first
            channel-min can start before the full image lands."""
            Xc = pool.tile([128, C * FD], BF16, tag="Xc", bufs=4, name=f"Xc_{who}")
            Xc4 = Xc.rearrange("p (ch c w) -> p ch c w", ch=C, w=W)
            src = inp[b].rearrange("ch (c p) w -> p ch c w", p=128)
            if split:
                nc.gpsimd.dma_start(Xc4[:, 0:2], src[:, 0:2])
                nc.gpsimd.dma_start(Xc4[:, 2:3], src[:, 2:3])
            else:
                nc.gpsimd.dma_start(Xc4, src)
            return Xc

        def maxpool_1d(Xp, out, who):
            """Sliding-window-35 max along the free dim of the padded
            [128, NCHUNK, NB] view Xp; writes [128, NCHUNK, W] into out.
            Zero pads double as -inf (all data >= 0)."""
            A = pool.tile([128, FDP], BF16, tag="A", bufs=2, name=f"A_{who}")
            Bt = pool.tile([128, FDP], BF16, tag="B", bufs=2, name=f"B_{who}")
            A3 = A.rearrange("p (c n) -> p c n", n=NB)
            B3 = Bt.rearrange("p (c n) -> p c n", n=NB)
            bufs = [Xp, A3, B3, A3, B3, A3]
            cov = 1  # current window length
            for j, s in enumerate(SHIFTS):
                src3 = bufs[j]
                dst3 = out if j == len(SHIFTS) - 1 else bufs[j + 1]
                span = W if j == len(SHIFTS) - 1 else NB - cov - s + 1
                nc.vector.tensor_tensor(
                    dst3[:, :, 0:span],
                    src3[:, :, 0:span],
                    src3[:, :, s : s + span],
                    ALU.max,
                )
                cov += s
            assert cov == K

        def w_phase(Xc, who, first):
            """channel-min + W-axis pool. Returns Rw [128, FD] bf16."""
            Xc3 = Xc.rearrange("p (ch n) -> p ch n", n=FD)
            nc.vector.tensor_tensor(Xc3[:, 0], Xc3[:, 0], Xc3[:, 1], ALU.min)
            X = pool.tile([128, FDP], BF16, tag="X", bufs=2, name=f"X_{who}")
            X3 = X.rearrange("p (c n) -> p c n", n=NB)
            if first:
                # pads sit at fixed slot addresses; later images reuse the
                # same two physical slots, whose pads stay zero
                nc.vector.memset(X3[:, :, 0:PAD], 0.0)
                nc.vector.memset(X3[:, :, H + PAD : NB], 0.0)
            nc.vector.tensor_tensor(
                X3[:, :, PAD : PAD + W],
                Xc3[:, 0].rearrange("p (c w) -> p c w", w=W),
                Xc3[:, 2].rearrange("p (c w) -> p c w", w=W),
                ALU.min,
            )
            Rw = pool.tile([128, FD], BF16, tag="Rw", bufs=4, name=f"Rw_{who}")
            maxpool_1d(X3, Rw.rearrange("p (c n) -> p c n", n=W), f"w{who}")
            return Rw

        def h_phase(Rw, who, first, ident):
            """PE transpose + H-axis pool. Returns RT [128, FD] bf16."""
            X2 = pool.tile([128, FDP], BF16, tag="X2", bufs=2, name=f"X2_{who}")
            X23 = X2.rearrange("p (d n) -> p d n", n=NB)
            if first:
                nc.vector.memset(X23[:, :, 0:PAD], 0.0)
                nc.vector.memset(X23[:, :, H + PAD : NB], 0.0)
            for d in range(NCHUNK):
                ps = ppool.tile([128, 512], BF16, tag="ps", name=f"ps_{who}_{d}")
                for c2 in range(NCHUNK):
                    nc.tensor.transpose(
                        ps[:, c2 * 128 : (c2 + 1) * 128],
                        Rw[:, c2 * W + d * 128 : c2 * W + d * 128 + 128],
                        ident[:],
                    )
                nc.scalar.copy(X23[:, d, PAD : PAD + H], ps[:])
            RT = pool.tile([128, FD], BF16, tag="RT", bufs=2, name=f"RT_{who}")
            maxpool_1d(X23, RT.rearrange("p (d n) -> p d n", n=H), f"h{who}")
            return RT

        smax = None
        for rep in range(reps):
            # issue all loads first so DMA streams ahead of compute
            Xcs = [
                load_image(inp, b, f"{nm}{b}_{rep}", split=(rep == 0 and b == 0))
                for b in range(B_PER_CORE)
                for nm, inp in (("r", restored), ("t", target))
            ]
            if rep == 0:
                ident = cpool.tile([128, 128], BF16)
                masks.make_identity(nc, ident[:])
                smax = cpool.tile([128, 1], F32)
                nc.vector.memset(smax[:], 0.0)

            first = rep == 0
            Rws = [
                w_phase(Xcs[i], f"i{i}_{rep}", sim_safe or (first and i < 2))
                for i in range(4)
            ]
            accs = []
            for p in range(B_PER_CORE):
                Rr = h_phase(
                    Rws[2 * p], f"hr{p}_{rep}", sim_safe or (first and p == 0), ident
                )
                Rt = h_phase(
                    Rws[2 * p + 1], f"ht{p}_{rep}", sim_safe or (first and p == 0), ident
                )
                scr = pool.tile([128, FD], BF16, tag="scr", bufs=2, name=f"scr{p}_{rep}")
                sabs = pool.tile([128, FD], BF16, tag="sabs", bufs=2, name=f"sabs{p}_{rep}")
                # halves: ACT's Abs+accum on half 0 overlaps DVE's diff on
                # half 1, shortening the exposed tail chain
                hf = FD // 2
                for hx in range(2):
                    amax = pool.tile(
                        [128, 1], F32, tag="amax", bufs=4, name=f"am{p}_{rep}_{hx}"
                    )
                    sl = slice(hx * hf, (hx + 1) * hf)
                    nc.vector.tensor_tensor(
                        scr[:, sl], Rr[:, sl], Rt[:, sl], ALU.subtract
                    )
                    nc.scalar.activation(
                        sabs[:, sl], scr[:, sl],
                        mybir.ActivationFunctionType.Abs, accum_out=amax[:],
                    )
                    accs.append(amax)
            for amax in accs:
                nc.vector.tensor_tensor(smax[:], smax[:], amax[:], ALU.add)

        out2 = pool.tile([128, 2], F32)
        nc.vector.memset(out2[:, 1:2], 0.0)
        nc.vector.tensor_copy(out2[:, 0:1], smax[:])
        nc.sync.dma_start(partial[:], out2[:])

    nc.compile()
    return nc


def _get_compiled():
    global _COMPILED
    if _COMPILED is None:
        _COMPILED = _build_nc()
    return _COMPILED


def kernel(restored: np.ndarray, target: np.ndarray) -> np.ndarray:
    restored = np.ascontiguousarray(restored, dtype=np.float32)
    target = np.ascontiguousarray(target, dtype=np.float32)
    nc = _get_compiled()
    in_maps = []
    for i in range(N_CORES):
        sl = slice(i * B_PER_CORE, (i + 1) * B_PER_CORE)
        in_maps.append(
            {
                "restored": np.ascontiguousarray(restored[sl]),
                "target": np.ascontiguousarray(target[sl]),
            }
        )
    res = run_bass_kernel_spmd(nc, in_maps, list(range(N_CORES)))
    total = np.float64(0.0)
    for r in res.results:
        p = np.asarray(r["partial"], dtype=np.float64)
        total += p[:, 0].sum() - p[:, 1].sum()
    mean = total / float(B_FULL * H * W)
    out = 1.0 / (1.0 + np.exp(-mean))
    return np.asarray(out, dtype=np.float32)
